# revision 16
# baseline (speedup 1.0000x reference)
"""Trainium2 Bass kernel for causal MLA self-attention.

Problem: B=2, T=2048, C=2048, H=16 heads, Dh=128, latent Dl=64.
  q = rope(x @ wq); k_lat = rope(x @ wk_lat); v_lat = x @ wv_lat
  k_h = k_lat @ k_expand[h]; v_h = v_lat @ v_expand[h]
  y = causal_softmax(q k^T / sqrt(Dh)) v;  out = y @ proj_w

Sharding: 8 cores = 2 batches x 4 head-groups (4 heads each).  Each core
computes a full (T, C) partial of the output projection restricted to its
heads; the host sums the 4 partials per batch.

Device algorithm (per core) uses full MLA absorption on both sides so
attention contracts over Dl=64 and the output projection reads the latent
attention output directly:
  qt_h = rope(q_h) @ k_expand[h]^T            (T, 64)
  s^T  = k_lat_rope @ qt_h^T                  (Tk, Tq) tiles, exp on ScalarE
  yu^T = [v_lat | 1]^T @ exp(s^T)             (65, Tq): row 64 = softmax denom
  yud  = yu / denom                           (64, Tq) bf16
  out += yud^T @ W_h,   W_h = v_expand[h] @ proj_w[head rows]  (host-folded)
All tensors are kept "transposed" (feature dim on partitions) so every
matmul contracts along partitions; softmax needs no max-subtraction
(scores are O(5)) and the denominator is a fused ones-column.  x, wq, wkv,
W and the output travel as bf16 to halve DMA; matmul inputs stay f32r on
the attention path.
"""

import os
import sys

import numpy as np

if not any(os.path.isdir(os.path.join(p, "concourse")) for p in sys.path if p):
    sys.path.insert(0, "/opt/trn_rl_repo")

import concourse.bass as bass  # noqa: E402
import concourse.bass_isa as bass_isa  # noqa: E402
import concourse.mybir as mybir  # noqa: E402
import concourse.tile as tile  # noqa: E402
from concourse import bacc  # noqa: E402
from concourse.bass_utils import run_bass_kernel_spmd  # noqa: E402

B, T, C, H, Dh, Dl = 2, 2048, 2048, 16, 128, 64
HPC = 4  # heads per core
NCORES = 8
F32 = mybir.dt.float32
F32R = mybir.dt.float32r
BF16 = mybir.dt.bfloat16
SCALE = 1.0 / float(np.sqrt(Dh))

TJ = 512          # Tq chunk (matmul moving-dim)
NJ = T // TJ      # 4
NK = C // 128     # 16 contraction chunks over C
NTK = T // 128    # 16 Tk chunks


def build_nc():
    nc = bacc.Bacc(None, target_bir_lowering=False, debug=False)

    xT = nc.dram_tensor("xT", [C, T], F32R, kind="ExternalInput")
    wq = nc.dram_tensor("wq", [C, HPC * Dh], F32R, kind="ExternalInput")
    wkv = nc.dram_tensor("wkv", [C, 2 * Dl], F32R, kind="ExternalInput")
    eT = nc.dram_tensor("eT", [Dh, HPC * Dl], F32R, kind="ExternalInput")
    eT2 = nc.dram_tensor("eT2", [Dh, HPC * Dl], F32R, kind="ExternalInput")
    wout = nc.dram_tensor("wout", [Dl, HPC, C], F32R, kind="ExternalInput")
    cosq = nc.dram_tensor("cosq", [Dh, T], BF16, kind="ExternalInput")
    sinq = nc.dram_tensor("sinq", [Dh, T], BF16, kind="ExternalInput")
    cosk = nc.dram_tensor("cosk", [Dl, T], BF16, kind="ExternalInput")
    sink = nc.dram_tensor("sink", [Dl, T], BF16, kind="ExternalInput")
    sperm = nc.dram_tensor("sperm", [Dl, Dl], F32R, kind="ExternalInput")
    ident = nc.dram_tensor("ident", [Dl, Dl], F32, kind="ExternalInput")
    maskt = nc.dram_tensor("maskt", [128, 4, TJ], BF16, kind="ExternalInput")
    onec = nc.dram_tensor("onec", [128, NTK], F32R, kind="ExternalInput")
    out = nc.dram_tensor("out", [T, C], BF16, kind="ExternalOutput")

    with tile.TileContext(nc) as tc, \
         nc.allow_low_precision(reason="bf16/f32r matmul pipeline"):
        consts = tc.alloc_tile_pool(name="consts", bufs=1)
        work = tc.alloc_tile_pool(name="work", bufs=1)

        # persistent cross-j tensors
        kk_sb = consts.tile([128, T], F32R, name="kk_sb")        # k_rope^T x2
        vaug_sb = consts.tile([128, NTK, Dl + 1], F32R, name="vaug_sb")
        wq_sb = consts.tile([128, NK, HPC * Dh], F32R, name="wq_sb")
        wkv_sb = consts.tile([128, NK, 2 * Dl], F32R, name="wkv_sb")
        eT_sb = consts.tile([Dh, HPC * Dl], F32R, name="eT_sb")
        eT2_sb = consts.tile([Dh, HPC * Dl], F32R, name="eT2_sb")
        cosq_sb = consts.tile([Dh, T], BF16, name="cosq_sb")
        sinq_sb = consts.tile([Dh, T], BF16, name="sinq_sb")
        cosk_sb = consts.tile([Dl, T], BF16, name="cosk_sb")
        sink_sb = consts.tile([Dl, T], BF16, name="sink_sb")
        sperm_sb = consts.tile([Dl, Dl], F32R, name="sperm_sb")
        ident_sb = consts.tile([Dl, Dl], F32, name="ident_sb")
        maskt_sb = consts.tile([128, 4, TJ], BF16, name="maskt_sb")
        wout_sb = consts.tile([Dl, HPC, C], F32R, name="wout_sb")

        with tc.tile_pool(name="psum", bufs=1, space="PSUM") as ps:
            # const DMAs needed by the j=0 kv/k-rope path first
            nc.sync.dma_start(sperm_sb, sperm[:])
            nc.sync.dma_start(ident_sb, ident[:])
            nc.sync.dma_start(
                wkv_sb, wkv[:].rearrange("(ko p) m -> p ko m", p=128))
            nc.sync.dma_start(cosk_sb, cosk[:])
            nc.sync.dma_start(sink_sb, sink[:])
            nc.sync.dma_start(vaug_sb[:, :, Dl:Dl + 1], onec[:, :, None])
            wq_r = wq[:].rearrange("(ko p) m -> p ko m", p=128)

            for j in range(NJ):
                js = slice(j * TJ, (j + 1) * TJ)
                # ---- stream x chunks (and, on j=0, the remaining consts) --
                xts = []
                for g in range(NK // 4):
                    xtg = work.tile([128, 4, TJ], F32R, name=f"xt{j}_{g}",
                                    tag="xt", bufs=5)
                    nc.sync.dma_start(
                        xtg, xT[g * 512:(g + 1) * 512, js].rearrange(
                            "(ko p) t -> p ko t", p=128))
                    if j == 0:
                        nc.sync.dma_start(wq_sb[:, 4 * g:4 * g + 4, :],
                                          wq_r[:, 4 * g:4 * g + 4, :])
                    xts.extend(xtg[:, i, :] for i in range(4))
                    nc.sync.dma_start(cosq_sb, cosq[:])
                    nc.sync.dma_start(sinq_sb, sinq[:])
                    nc.sync.dma_start(eT_sb, eT[:])
                    nc.sync.dma_start(eT2_sb, eT2[:])
                    nc.sync.dma_start(maskt_sb, maskt[:])
                    nc.sync.dma_start(wout_sb, wout[:])

                # ---- latent kv projection + k RoPE ----------------------
                kvps = ps.tile([128, TJ], F32, name=f"kvps{j}", tag="proj",
                               bufs=2)
                for k in range(NK):
                    nc.tensor.matmul(kvps, wkv_sb[:, k, :], xts[k],
                                     start=(k == 0), stop=(k == NK - 1))
                klat = work.tile([Dl, TJ], F32R, name=f"klat{j}", tag="klat",
                                 bufs=1)
                nc.vector.tensor_copy(klat, kvps[0:Dl, :])
                vT = work.tile([Dl, TJ], F32, name=f"vT{j}", tag="vT",
                               bufs=1)
                nc.scalar.copy(vT, kvps[Dl:128, :])
                ksps = ps.tile([Dl, TJ], F32, name=f"ksps{j}", tag="proj",
                               bufs=2)
                nc.tensor.matmul(ksps, sperm_sb, klat, start=True, stop=True)
                tk1 = work.tile([Dl, TJ], F32R, name=f"tk1_{j}", tag="tk1",
                                bufs=1)
                nc.gpsimd.tensor_mul(tk1, klat, cosk_sb[:, js])
                tk2 = work.tile([Dl, TJ], F32R, name=f"tk2_{j}", tag="tk2",
                                bufs=1)
                nc.vector.tensor_mul(tk2, ksps, sink_sb[:, js])
                nc.vector.tensor_add(kk_sb[0:Dl, js], tk1, tk2)
                nc.scalar.copy(kk_sb[Dl:128, js], kk_sb[0:Dl, js])

                # ---- v_lat -> natural layout [v | 1] tiles --------------
                for n in range(4 * j, 4 * j + 4):
                    vtp = ps.tile([128, Dl], F32, name=f"vtp{n}", tag="proj",
                                  bufs=2)
                    nc.tensor.transpose(
                        vtp, vT[:, (n - 4 * j) * 128:(n - 4 * j + 1) * 128],
                        ident_sb)
                    nc.vector.tensor_copy(vaug_sb[:, n, 0:Dl], vtp)

                # ---- q heads: project + fused RoPE/absorb ---------------
                # qt = E (q*cos) + (E S) (q*sin_swapped)
                qtil = work.tile([128, 2, TJ], F32R, name=f"qtil{j}",
                                 tag="qtil", bufs=2)
                for m in range(HPC):
                    qp = ps.tile([128, TJ], F32, name=f"qps{j}_{m}",
                                 tag="proj", bufs=2)
                    for k in range(NK):
                        nc.tensor.matmul(
                            qp, wq_sb[:, k, m * 128:(m + 1) * 128], xts[k],
                            start=(k == 0), stop=(k == NK - 1))
                    u1 = work.tile([128, TJ], F32R, name=f"u1_{j}_{m}",
                                   tag="u1", bufs=1)
                    nc.vector.tensor_mul(u1, qp, cosq_sb[:, js])
                    u2 = work.tile([128, TJ], F32R, name=f"u2_{j}_{m}",
                                   tag="u2", bufs=2)
                    nc.vector.tensor_mul(u2, qp, sinq_sb[:, js])
                    p, half = divmod(m, 2)
                    qtp = ps.tile([Dl, TJ], F32, name=f"qtp{j}_{m}",
                                  tag="proj", bufs=2)
                    msl = slice(m * Dl, (m + 1) * Dl)
                    nc.tensor.matmul(qtp, eT_sb[:, msl], u1,
                                     start=True, stop=False)
                    nc.tensor.matmul(qtp, eT2_sb[:, msl], u2,
                                     start=False, stop=True)
                    nc.vector.tensor_copy(
                        qtil[half * Dl:(half + 1) * Dl, p, :], qtp)

                # ---- attention (per head), latent-absorbed --------------
                yud = work.tile([Dl, HPC, TJ], F32R, name=f"yud{j}",
                                tag="yud", bufs=2)
                for h in range(HPC):
                    p, half = divmod(h, 2)
                    qrow = slice(half * Dl, (half + 1) * Dl)
                    nm = 4 * (j + 1)
                    avt = ps.tile([Dl + 1, TJ], F32, name=f"av{j}_{h}",
                                  tag="av", bufs=2)
                    pend = None  # software-pipeline AV one step behind
                    for m in range(nm):
                        ms = slice(m * 128, (m + 1) * 128)
                        d = m - 4 * j if m >= 4 * j else -1
                        lo = 128 * d if d > 0 else 0
                        jsl = slice(lo, TJ)
                        sps = ps.tile([128, TJ], F32, name=f"sps{j}_{h}_{m}",
                                      tag="s", bufs=2)
                        nc.tensor.matmul(sps[:, jsl], kk_sb[qrow, ms],
                                         qtil[qrow, p, jsl],
                                         start=True, stop=True,
                                         tile_position=(half * Dl, 0))
                        ex = work.tile([128, TJ], F32R,
                                       name=f"ex{j}_{h}_{m}", tag="ex",
                                       bufs=3)
                        nc.scalar.activation(
                            ex[:, jsl], sps[:, jsl],
                            mybir.ActivationFunctionType.Exp, scale=SCALE)
                        if d >= 0:
                            # mask only the 128-wide staircase band
                            band = slice(lo, lo + 128)
                            nc.gpsimd.tensor_mul(
                                ex[:, band], ex[:, band],
                                maskt_sb[:, d, band])
                        if pend is not None:
                            plo, plhs, pex = pend
                            nc.tensor.matmul(avt[:, plo:], plhs,
                                             pex[:, plo:],
                                             start=(m == 1), stop=False)
                        pend = (lo, vaug_sb[:, m, :], ex)
                    plo, plhs, pex = pend
                    nc.tensor.matmul(avt[:, plo:], plhs, pex[:, plo:],
                                     start=False, stop=True)
                    # divide by the softmax denominator (row Dl of avt)
                    rin = work.tile([1, TJ], F32R, name=f"rin{j}_{h}",
                                    tag="rin", bufs=2)
                    nc.vector.reciprocal(rin, avt[Dl:Dl + 1, :])
                    rbb = work.tile([Dl, TJ], F32R, name=f"rbb{j}_{h}",
                                    tag="rbb", bufs=1)
                    nc.gpsimd.partition_broadcast(rbb, rin, channels=Dl)
                    nc.vector.tensor_mul(yud[:, h, :], avt[0:Dl, :], rbb)

                # ---- output projection for this j's Tq rows -------------
                out_r = out[:].rearrange("(mi p) (n t) -> mi p n t",
                                         p=128, n=NJ)
                for mi in range(4 * j, 4 * j + 4):
                    lsl = slice((mi - 4 * j) * 128, (mi - 4 * j + 1) * 128)
                    ot = work.tile([128, NJ, TJ], BF16, name=f"ot{mi}",
                                   tag="ot", bufs=2)
                    for n in range(NJ):
                        pps = ps.tile([128, TJ], F32, name=f"pps{mi}_{n}",
                                      tag="pp", bufs=2)
                        for k in range(HPC):
                            nc.tensor.matmul(
                                pps, yud[:, k, lsl],
                                wout_sb[:, k, n * TJ:(n + 1) * TJ],
                                start=(k == 0), stop=(k == HPC - 1))
                        nc.vector.tensor_copy(ot[:, n, :], pps)
                    nc.sync.dma_start(out_r[mi], ot)

        work.release()
        consts.release()

    nc.compile()
    return nc


def _rope_tables(t, d):
    inv = 1.0 / (10000.0 ** (np.arange(0, d, 2, dtype=np.float64) / d))
    ang = np.arange(t, dtype=np.float64)[:, None] * inv[None, :]  # (t, d/2)
    cos = np.cos(ang).T  # (d/2, t)
    sin = np.sin(ang).T
    cosf = np.empty((d, t), np.float32)
    sinf = np.empty((d, t), np.float32)
    cosf[0::2] = cos
    cosf[1::2] = cos
    sinf[0::2] = -sin
    sinf[1::2] = sin
    return cosf, sinf


def _host_inputs(x, wq, wk_lat, wv_lat, k_expand, v_expand, proj_w):
    import ml_dtypes
    bf = ml_dtypes.bfloat16
    cosq, sinq = _rope_tables(T, Dh)
    sinq = np.ascontiguousarray(sinq[np.arange(Dh) ^ 1, :])  # row-pair swap
    cosk, sink = _rope_tables(T, Dl)
    idx = np.arange(Dl)
    sperm = np.zeros((Dl, Dl), np.float32)
    sperm[idx, idx ^ 1] = 1.0
    ident = np.eye(Dl, dtype=np.float32)
    tkr = np.arange(128)[:, None]
    tqr = np.arange(TJ)[None, :]
    maskt = np.stack(
        [(tkr + 128 * d <= tqr).astype(bf) for d in range(4)], axis=1)
    wkv = np.ascontiguousarray(np.concatenate([wk_lat, wv_lat], axis=1))

    xTs = [np.ascontiguousarray(x[b].T) for b in range(B)]
    hidx = np.arange(Dh)
    in_maps = []
    for core in range(NCORES):
        b, g = divmod(core, 4)
        heads = range(4 * g, 4 * g + 4)
        eTc = np.ascontiguousarray(
            np.concatenate([k_expand[h].T for h in heads], axis=1))
        eT2c = np.ascontiguousarray(eTc[hidx ^ 1, :])
        # fold v_expand into the output projection: W_h = v_expand[h] @ P_h
        wout = np.stack(
            [(v_expand[h].astype(np.float64)
              @ proj_w[h * Dh:(h + 1) * Dh].astype(np.float64)
              ).astype(np.float32) for h in heads], axis=1)  # (Dl, HPC, C)
        in_maps.append({
            "xT": xTs[b],
            "wq": np.ascontiguousarray(wq[:, g * 512:(g + 1) * 512]),
            "wkv": wkv,
            "eT": eTc, "eT2": eT2c,
            "wout": wout,
            "cosq": cosq.astype(bf), "sinq": sinq.astype(bf),
            "cosk": cosk.astype(bf), "sink": sink.astype(bf),
            "sperm": sperm, "ident": ident, "maskt": maskt,
            "onec": np.ones((128, NTK), np.float32),
        })
    return in_maps


_NC_CACHE = {}


def run(inputs, trace=False, **kw):
    """Run on all 8 cores; returns (output, BassKernelResults)."""
    if "nc" not in _NC_CACHE:
        _NC_CACHE["nc"] = build_nc()
    nc = _NC_CACHE["nc"]
    in_maps = _host_inputs(**inputs)
    res = run_bass_kernel_spmd(
        nc, in_maps, core_ids=list(range(NCORES)), trace=trace, **kw)
    out = np.zeros((B, T, C), np.float32)
    for core in range(NCORES):
        out[core // 4] += res.results[core]["out"].astype(np.float32)
    return out, res


def kernel(**inputs):
    out, _ = run(inputs)
    return out


# revision 17
# speedup vs baseline: 1.0383x; 1.0383x over previous
"""Trainium2 Bass kernel for causal MLA self-attention.

Problem: B=2, T=2048, C=2048, H=16 heads, Dh=128, latent Dl=64.
  q = rope(x @ wq); k_lat = rope(x @ wk_lat); v_lat = x @ wv_lat
  k_h = k_lat @ k_expand[h]; v_h = v_lat @ v_expand[h]
  y = causal_softmax(q k^T / sqrt(Dh)) v;  out = y @ proj_w

Sharding: 8 cores = 2 batches x 4 head-groups (4 heads each).  Each core
computes a full (T, C) partial of the output projection restricted to its
heads; the host sums the 4 partials per batch.

Device algorithm (per core) uses full MLA absorption on both sides so
attention contracts over Dl=64 and the output projection reads the latent
attention output directly:
  qt_h = rope(q_h) @ k_expand[h]^T            (T, 64)
  s^T  = k_lat_rope @ qt_h^T                  (Tk, Tq) tiles, exp on ScalarE
  yu^T = [v_lat | 1]^T @ exp(s^T)             (65, Tq): row 64 = softmax denom
  yud  = yu / denom                           (64, Tq) bf16
  out += yud^T @ W_h,   W_h = v_expand[h] @ proj_w[head rows]  (host-folded)
All tensors are kept "transposed" (feature dim on partitions) so every
matmul contracts along partitions; softmax needs no max-subtraction
(scores are O(5)) and the denominator is a fused ones-column.  x, wq, wkv,
W and the output travel as bf16 to halve DMA; matmul inputs stay f32r on
the attention path.
"""

import os
import sys

import numpy as np

if not any(os.path.isdir(os.path.join(p, "concourse")) for p in sys.path if p):
    sys.path.insert(0, "/opt/trn_rl_repo")

import concourse.bass as bass  # noqa: E402
import concourse.bass_isa as bass_isa  # noqa: E402
import concourse.mybir as mybir  # noqa: E402
import concourse.tile as tile  # noqa: E402
from concourse import bacc  # noqa: E402
from concourse.bass_utils import run_bass_kernel_spmd  # noqa: E402

B, T, C, H, Dh, Dl = 2, 2048, 2048, 16, 128, 64
HPC = 4  # heads per core
NCORES = 8
F32 = mybir.dt.float32
F32R = mybir.dt.float32r
BF16 = mybir.dt.bfloat16
SCALE = 1.0 / float(np.sqrt(Dh))

TJ = 512          # Tq chunk (matmul moving-dim)
NJ = T // TJ      # 4
NK = C // 128     # 16 contraction chunks over C
NTK = T // 128    # 16 Tk chunks


def build_nc():
    nc = bacc.Bacc(None, target_bir_lowering=False, debug=False)

    xT = nc.dram_tensor("xT", [C, T], F32R, kind="ExternalInput")
    wq = nc.dram_tensor("wq", [C, HPC * Dh], F32R, kind="ExternalInput")
    wkv = nc.dram_tensor("wkv", [C, 2 * Dl], F32R, kind="ExternalInput")
    eT = nc.dram_tensor("eT", [Dh, HPC * Dl], F32R, kind="ExternalInput")
    eT2 = nc.dram_tensor("eT2", [Dh, HPC * Dl], F32R, kind="ExternalInput")
    wout = nc.dram_tensor("wout", [Dl, HPC, C], F32R, kind="ExternalInput")
    cosq = nc.dram_tensor("cosq", [Dh, T], BF16, kind="ExternalInput")
    sinq = nc.dram_tensor("sinq", [Dh, T], BF16, kind="ExternalInput")
    cosk = nc.dram_tensor("cosk", [Dl, T], BF16, kind="ExternalInput")
    sink = nc.dram_tensor("sink", [Dl, T], BF16, kind="ExternalInput")
    sperm = nc.dram_tensor("sperm", [Dl, Dl], F32R, kind="ExternalInput")
    ident = nc.dram_tensor("ident", [Dl, Dl], F32, kind="ExternalInput")
    maskt = nc.dram_tensor("maskt", [128, 4, TJ], BF16, kind="ExternalInput")
    onec = nc.dram_tensor("onec", [128, NTK], F32R, kind="ExternalInput")
    out = nc.dram_tensor("out", [T, C], BF16, kind="ExternalOutput")

    with tile.TileContext(nc) as tc, \
         nc.allow_low_precision(reason="bf16/f32r matmul pipeline"):
        consts = tc.alloc_tile_pool(name="consts", bufs=1)
        work = tc.alloc_tile_pool(name="work", bufs=1)

        # persistent cross-j tensors
        kk_sb = consts.tile([128, T], F32R, name="kk_sb")        # k_rope^T x2
        vaug_sb = consts.tile([128, NTK, Dl + 1], F32R, name="vaug_sb")
        wq_sb = consts.tile([128, NK, HPC * Dh], F32R, name="wq_sb")
        wkv_sb = consts.tile([128, NK, 2 * Dl], F32R, name="wkv_sb")
        eT_sb = consts.tile([Dh, HPC * Dl], F32R, name="eT_sb")
        eT2_sb = consts.tile([Dh, HPC * Dl], F32R, name="eT2_sb")
        cosq_sb = consts.tile([Dh, T], BF16, name="cosq_sb")
        sinq_sb = consts.tile([Dh, T], BF16, name="sinq_sb")
        cosk_sb = consts.tile([Dl, T], BF16, name="cosk_sb")
        sink_sb = consts.tile([Dl, T], BF16, name="sink_sb")
        sperm_sb = consts.tile([Dl, Dl], F32R, name="sperm_sb")
        ident_sb = consts.tile([Dl, Dl], F32, name="ident_sb")
        maskt_sb = consts.tile([128, 4, TJ], BF16, name="maskt_sb")
        wout_sb = consts.tile([Dl, HPC, C], F32R, name="wout_sb")

        with tc.tile_pool(name="psum", bufs=1, space="PSUM") as ps:
            # const DMAs needed by the j=0 kv/k-rope path first
            nc.sync.dma_start(sperm_sb, sperm[:])
            nc.sync.dma_start(ident_sb, ident[:])
            nc.sync.dma_start(
                wkv_sb, wkv[:].rearrange("(ko p) m -> p ko m", p=128))
            nc.sync.dma_start(cosk_sb, cosk[:])
            nc.sync.dma_start(sink_sb, sink[:])
            nc.sync.dma_start(vaug_sb[:, :, Dl:Dl + 1], onec[:, :, None])
            wq_r = wq[:].rearrange("(ko p) m -> p ko m", p=128)

            for j in range(NJ):
                js = slice(j * TJ, (j + 1) * TJ)
                # ---- stream x chunks (and, on j=0, the remaining consts) --
                xts = []
                for g in range(NK // 4):
                    xtg = work.tile([128, 4, TJ], F32R, name=f"xt{j}_{g}",
                                    tag="xt", bufs=4)
                    nc.sync.dma_start(
                        xtg, xT[g * 512:(g + 1) * 512, js].rearrange(
                            "(ko p) t -> p ko t", p=128))
                    if j == 0:
                        nc.sync.dma_start(wq_sb[:, 4 * g:4 * g + 4, :],
                                          wq_r[:, 4 * g:4 * g + 4, :])
                    xts.extend(xtg[:, i, :] for i in range(4))
                    nc.sync.dma_start(cosq_sb, cosq[:])
                    nc.sync.dma_start(sinq_sb, sinq[:])
                    nc.sync.dma_start(eT_sb, eT[:])
                    nc.sync.dma_start(eT2_sb, eT2[:])
                    nc.sync.dma_start(maskt_sb, maskt[:])
                    nc.sync.dma_start(wout_sb, wout[:])

                # ---- latent kv projection + k RoPE ----------------------
                kvps = ps.tile([128, TJ], F32, name=f"kvps{j}", tag="proj",
                               bufs=2)
                for k in range(NK):
                    nc.tensor.matmul(kvps, wkv_sb[:, k, :], xts[k],
                                     start=(k == 0), stop=(k == NK - 1))
                klat = work.tile([Dl, TJ], F32R, name=f"klat{j}", tag="klat",
                                 bufs=1)
                nc.vector.tensor_copy(klat, kvps[0:Dl, :])
                vT = work.tile([Dl, TJ], F32, name=f"vT{j}", tag="vT",
                               bufs=1)
                nc.scalar.copy(vT, kvps[Dl:128, :])
                ksps = ps.tile([Dl, TJ], F32, name=f"ksps{j}", tag="proj",
                               bufs=2)
                nc.tensor.matmul(ksps, sperm_sb, klat, start=True, stop=True)
                tk1 = work.tile([Dl, TJ], F32R, name=f"tk1_{j}", tag="tk1",
                                bufs=1)
                nc.gpsimd.tensor_mul(tk1, klat, cosk_sb[:, js])
                tk2 = work.tile([Dl, TJ], F32R, name=f"tk2_{j}", tag="tk2",
                                bufs=1)
                nc.vector.tensor_mul(tk2, ksps, sink_sb[:, js])
                nc.vector.tensor_add(kk_sb[0:Dl, js], tk1, tk2)
                nc.scalar.copy(kk_sb[Dl:128, js], kk_sb[0:Dl, js])

                # ---- v_lat -> natural layout [v | 1] tiles --------------
                for n in range(4 * j, 4 * j + 4):
                    vtp = ps.tile([128, Dl], F32, name=f"vtp{n}", tag="proj",
                                  bufs=2)
                    nc.tensor.transpose(
                        vtp, vT[:, (n - 4 * j) * 128:(n - 4 * j + 1) * 128],
                        ident_sb)
                    nc.vector.tensor_copy(vaug_sb[:, n, 0:Dl], vtp)

                # ---- q heads: project + fused RoPE/absorb ---------------
                # qt = E (q*cos) + (E S) (q*sin_swapped)
                qtil = work.tile([128, 2, TJ], F32R, name=f"qtil{j}",
                                 tag="qtil", bufs=2)
                for m in range(HPC):
                    qp = ps.tile([128, TJ], F32, name=f"qps{j}_{m}",
                                 tag="proj", bufs=2)
                    for k in range(NK):
                        nc.tensor.matmul(
                            qp, wq_sb[:, k, m * 128:(m + 1) * 128], xts[k],
                            start=(k == 0), stop=(k == NK - 1))
                    u1 = work.tile([128, TJ], F32R, name=f"u1_{j}_{m}",
                                   tag="u1", bufs=1)
                    nc.vector.tensor_mul(u1, qp, cosq_sb[:, js])
                    u2 = work.tile([128, TJ], F32R, name=f"u2_{j}_{m}",
                                   tag="u2", bufs=2)
                    nc.vector.tensor_mul(u2, qp, sinq_sb[:, js])
                    p, half = divmod(m, 2)
                    qtp = ps.tile([Dl, TJ], F32, name=f"qtp{j}_{m}",
                                  tag="proj", bufs=2)
                    msl = slice(m * Dl, (m + 1) * Dl)
                    nc.tensor.matmul(qtp, eT_sb[:, msl], u1,
                                     start=True, stop=False)
                    nc.tensor.matmul(qtp, eT2_sb[:, msl], u2,
                                     start=False, stop=True)
                    nc.vector.tensor_copy(
                        qtil[half * Dl:(half + 1) * Dl, p, :], qtp)

                # ---- attention (per head), latent-absorbed --------------
                yud = work.tile([Dl, HPC, TJ], F32R, name=f"yud{j}",
                                tag="yud", bufs=2)
                for h in range(HPC):
                    p, half = divmod(h, 2)
                    qrow = slice(half * Dl, (half + 1) * Dl)
                    nm = 4 * (j + 1)
                    avt = ps.tile([Dl + 1, TJ], F32, name=f"av{j}_{h}",
                                  tag="av", bufs=2)
                    pend = None  # software-pipeline AV one step behind
                    for m in range(nm):
                        ms = slice(m * 128, (m + 1) * 128)
                        d = m - 4 * j if m >= 4 * j else -1
                        lo = 128 * d if d > 0 else 0
                        jsl = slice(lo, TJ)
                        sps = ps.tile([128, TJ], F32, name=f"sps{j}_{h}_{m}",
                                      tag="s", bufs=2)
                        nc.tensor.matmul(sps[:, jsl], kk_sb[qrow, ms],
                                         qtil[qrow, p, jsl],
                                         start=True, stop=True,
                                         tile_position=(half * Dl, 0))
                        ex = work.tile([128, TJ], F32R,
                                       name=f"ex{j}_{h}_{m}", tag="ex",
                                       bufs=5)
                        nc.scalar.activation(
                            ex[:, jsl], sps[:, jsl],
                            mybir.ActivationFunctionType.Exp, scale=SCALE)
                        if d >= 0:
                            # mask only the 128-wide staircase band
                            band = slice(lo, lo + 128)
                            nc.gpsimd.tensor_mul(
                                ex[:, band], ex[:, band],
                                maskt_sb[:, d, band])
                        if pend is not None:
                            plo, plhs, pex = pend
                            nc.tensor.matmul(avt[:, plo:], plhs,
                                             pex[:, plo:],
                                             start=(m == 1), stop=False)
                        pend = (lo, vaug_sb[:, m, :], ex)
                    plo, plhs, pex = pend
                    nc.tensor.matmul(avt[:, plo:], plhs, pex[:, plo:],
                                     start=False, stop=True)
                    # divide by the softmax denominator (row Dl of avt)
                    rin = work.tile([1, TJ], F32R, name=f"rin{j}_{h}",
                                    tag="rin", bufs=2)
                    nc.vector.reciprocal(rin, avt[Dl:Dl + 1, :])
                    rbb = work.tile([Dl, TJ], F32R, name=f"rbb{j}_{h}",
                                    tag="rbb", bufs=1)
                    nc.gpsimd.partition_broadcast(rbb, rin, channels=Dl)
                    nc.vector.tensor_mul(yud[:, h, :], avt[0:Dl, :], rbb)

                # ---- output projection for this j's Tq rows -------------
                out_r = out[:].rearrange("(mi p) (n t) -> mi p n t",
                                         p=128, n=NJ)
                for mi in range(4 * j, 4 * j + 4):
                    lsl = slice((mi - 4 * j) * 128, (mi - 4 * j + 1) * 128)
                    ot = work.tile([128, NJ, TJ], BF16, name=f"ot{mi}",
                                   tag="ot", bufs=2)
                    for n in range(NJ):
                        pps = ps.tile([128, TJ], F32, name=f"pps{mi}_{n}",
                                      tag="pp", bufs=2)
                        for k in range(HPC):
                            nc.tensor.matmul(
                                pps, yud[:, k, lsl],
                                wout_sb[:, k, n * TJ:(n + 1) * TJ],
                                start=(k == 0), stop=(k == HPC - 1))
                        nc.vector.tensor_copy(ot[:, n, :], pps)
                    nc.sync.dma_start(out_r[mi], ot)

        work.release()
        consts.release()

    nc.compile()
    return nc


def _rope_tables(t, d):
    inv = 1.0 / (10000.0 ** (np.arange(0, d, 2, dtype=np.float64) / d))
    ang = np.arange(t, dtype=np.float64)[:, None] * inv[None, :]  # (t, d/2)
    cos = np.cos(ang).T  # (d/2, t)
    sin = np.sin(ang).T
    cosf = np.empty((d, t), np.float32)
    sinf = np.empty((d, t), np.float32)
    cosf[0::2] = cos
    cosf[1::2] = cos
    sinf[0::2] = -sin
    sinf[1::2] = sin
    return cosf, sinf


def _host_inputs(x, wq, wk_lat, wv_lat, k_expand, v_expand, proj_w):
    import ml_dtypes
    bf = ml_dtypes.bfloat16
    cosq, sinq = _rope_tables(T, Dh)
    sinq = np.ascontiguousarray(sinq[np.arange(Dh) ^ 1, :])  # row-pair swap
    cosk, sink = _rope_tables(T, Dl)
    idx = np.arange(Dl)
    sperm = np.zeros((Dl, Dl), np.float32)
    sperm[idx, idx ^ 1] = 1.0
    ident = np.eye(Dl, dtype=np.float32)
    tkr = np.arange(128)[:, None]
    tqr = np.arange(TJ)[None, :]
    maskt = np.stack(
        [(tkr + 128 * d <= tqr).astype(bf) for d in range(4)], axis=1)
    wkv = np.ascontiguousarray(np.concatenate([wk_lat, wv_lat], axis=1))

    xTs = [np.ascontiguousarray(x[b].T) for b in range(B)]
    hidx = np.arange(Dh)
    in_maps = []
    for core in range(NCORES):
        b, g = divmod(core, 4)
        heads = range(4 * g, 4 * g + 4)
        eTc = np.ascontiguousarray(
            np.concatenate([k_expand[h].T for h in heads], axis=1))
        eT2c = np.ascontiguousarray(eTc[hidx ^ 1, :])
        # fold v_expand into the output projection: W_h = v_expand[h] @ P_h
        wout = np.stack(
            [(v_expand[h].astype(np.float64)
              @ proj_w[h * Dh:(h + 1) * Dh].astype(np.float64)
              ).astype(np.float32) for h in heads], axis=1)  # (Dl, HPC, C)
        in_maps.append({
            "xT": xTs[b],
            "wq": np.ascontiguousarray(wq[:, g * 512:(g + 1) * 512]),
            "wkv": wkv,
            "eT": eTc, "eT2": eT2c,
            "wout": wout,
            "cosq": cosq.astype(bf), "sinq": sinq.astype(bf),
            "cosk": cosk.astype(bf), "sink": sink.astype(bf),
            "sperm": sperm, "ident": ident, "maskt": maskt,
            "onec": np.ones((128, NTK), np.float32),
        })
    return in_maps


_NC_CACHE = {}


def run(inputs, trace=False, **kw):
    """Run on all 8 cores; returns (output, BassKernelResults)."""
    if "nc" not in _NC_CACHE:
        _NC_CACHE["nc"] = build_nc()
    nc = _NC_CACHE["nc"]
    in_maps = _host_inputs(**inputs)
    res = run_bass_kernel_spmd(
        nc, in_maps, core_ids=list(range(NCORES)), trace=trace, **kw)
    out = np.zeros((B, T, C), np.float32)
    for core in range(NCORES):
        out[core // 4] += res.results[core]["out"].astype(np.float32)
    return out, res


def kernel(**inputs):
    out, _ = run(inputs)
    return out
